# revision 20
# baseline (speedup 1.0000x reference)
"""DeepseekMoE (moe_routing) Trainium2 kernel.

Strategy (8 NeuronCores, single SPMD program):
  - Routing (grouped top-k) runs on host in numpy.
  - Routed experts are piece-parallel: the token list of each expert is
    split into pieces so that per-core slot capacities stay small even
    when routing is skewed (hot experts span two cores).  Every core
    gets one "big" piece (capacity C1 ~ 400) and one "small" piece
    (capacity C2 ~ 176); weights are per-(core,slot) inputs so two
    cores can hold the same expert.  Tiny experts (<=64 tokens, <=128
    total) are evaluated on the host and added during combine.
  - Shared expert MLP is tensor-parallel over the 8 cores along the
    intermediate dim (2816 -> 8 x 352, zero-padded to 8 x 384).
  - All matmul operands are bf16 (cast host-side), accumulation f32;
    device outputs are bf16 (summed in f32 on host).
  - Device phases are interleaved so weight-panel DMA, PSUM eviction
    and TensorE work overlap: slot0 GEMM1 -> [slot0 GEMM2 | slot1
    GEMM1 | shared GEMM1] -> [shared GEMM2 | slot1 GEMM2].
"""

import numpy as np
import ml_dtypes

import concourse.mybir as mybir
import concourse.tile as tile
from concourse import bacc
from concourse.bass_utils import run_bass_kernel_spmd
from concourse.engine_type import EngineType

BF16 = ml_dtypes.bfloat16
F32 = np.float32

# Problem shapes (fixed by the spec).
T, H, E, I = 1024, 2048, 16, 1408
I2 = 2 * I                      # 2816 (w13 rows per expert)
IS = 2 * I                      # shared intermediate (n_shared=2 -> 2816)
SSH = 384                       # per-core shared shard (2816 padded to 3072 = 8*384)
TOP_K, N_GROUP, TOPK_GROUP = 4, 4, 2
ROUTED_SCALE = 2.5
N_CORES = 8
P = 128
KH = H // P                     # 16 K-subtiles over H
KI = I // P                     # 11 K-subtiles over I
MW = I2 // P                    # 22 M-panels over 2I
MH = H // P                     # 16 M-panels over H
NPAIR = I // P                  # 11 (g,u) SwiGLU pairs per slot
KS = SSH // P                   # 3 K-subtiles over shared shard

HOST_EXPERT_MAX = 64            # experts this small may be host-computed
HOST_TOKEN_BUDGET = 128         # total tokens allowed on host


def _sigmoid(x):
    return 1.0 / (1.0 + np.exp(-x))


def _route(x, gate_weight, gate_bias):
    """Numpy port of reference._grouped_topk (float64 internally)."""
    logits = x.astype(np.float64) @ gate_weight.astype(np.float64).T
    scores = _sigmoid(logits)
    choice = scores + gate_bias.astype(np.float64)[None, :]
    g = choice.reshape(T, N_GROUP, E // N_GROUP)
    top2sum = np.sort(g, axis=-1)[..., -2:].sum(-1)          # [T, NG]
    gidx = np.argsort(-top2sum, axis=-1, kind="stable")[:, :TOPK_GROUP]
    gmask = np.zeros((T, N_GROUP), bool)
    gmask[np.arange(T)[:, None], gidx] = True
    emask = np.repeat(gmask, E // N_GROUP, axis=1)           # [T, E]
    masked = np.where(emask, choice, -np.inf)
    topk_ids = np.argsort(-masked, axis=-1, kind="stable")[:, :TOP_K]
    topk_w = np.take_along_axis(scores, topk_ids, axis=1)
    topk_w = topk_w / topk_w.sum(-1, keepdims=True) * ROUTED_SCALE
    return topk_ids.astype(np.int32), topk_w


def _pack_lhs_panels(w, n_m, n_k):
    """[n_m*128, n_k*128] (indexed [M, K]) -> [n_m, 128, n_k, 128] panels
    where panel[m][p, k, j] = w[128*m + j, 128*k + p], i.e. each panel
    slice [:, k, :] is the lhsT chunk [K-sub=128, M-sub=128]."""
    a = w.reshape(n_m, P, n_k, P)          # [m, j, k, p]
    return np.ascontiguousarray(a.transpose(0, 3, 2, 1))


def _pack_rhs(xcols):
    """[C, H] token-major rows -> [128, KH, C] rhs layout:
    out[p, k, c] = xcols[c, 128*k + p]."""
    a = xcols.reshape(-1, KH, P)           # [c, k, p]
    return np.ascontiguousarray(a.transpose(2, 1, 0))


def _rnd8(n):
    return max(8, int(-(-n // 8)) * 8) if n > 0 else 0


def _plan(counts):
    """Decide host-computed experts and (expert, lo, hi) pieces with a
    per-core 2-slot layout.  Returns (host_experts, slot1, slot2, C1, C2)
    where slotK is a list of length N_CORES of pieces or None."""
    # 1. host set: smallest experts first, bounded token budget
    host = []
    tot = 0
    for e in np.argsort(counts, kind="stable"):
        c = int(counts[e])
        if c == 0 or c > HOST_EXPERT_MAX:
            continue
        if tot + c <= HOST_TOKEN_BUDGET:
            host.append(int(e))
            tot += c
    host = set(host)

    pieces = [(int(e), 0, int(counts[e]))
              for e in range(E) if counts[e] > 0 and e not in host]
    total = sum(hi - lo for _, lo, hi in pieces)
    if total == 0:
        return host, [None] * N_CORES, [None] * N_CORES, 0, 0
    avg16 = max(1, total // (2 * N_CORES))

    def plen(p):
        return p[2] - p[1]

    def split_at(ps, i):
        e, lo, hi = ps[i]
        mid = lo + (hi - lo + 1) // 2
        return ps[:i] + [(e, lo, mid), (e, mid, hi)] + ps[i + 1:]

    # 2a. split anything clearly oversized
    while len(pieces) < 2 * N_CORES:
        i = max(range(len(pieces)), key=lambda j: plen(pieces[j]))
        if plen(pieces[i]) > 1.6 * avg16:
            pieces = split_at(pieces, i)
        else:
            break

    # 2b. greedy cost-reducing splits while slots remain
    def caps_of(ps):
        ln = sorted((plen(p) for p in ps), reverse=True)
        c1 = _rnd8(max(ln[:N_CORES]))
        c2 = _rnd8(max(ln[N_CORES:2 * N_CORES])) if len(ln) > N_CORES else 0
        return c1 + c2

    while len(pieces) < 2 * N_CORES:
        cur = caps_of(pieces)
        best_gain, best_i = 0, None
        for i in range(len(pieces)):
            if plen(pieces[i]) < 16:
                continue
            gain = cur - caps_of(split_at(pieces, i))
            if gain > best_gain:
                best_gain, best_i = gain, i
        if best_i is None:
            break
        pieces = split_at(pieces, best_i)

    pieces.sort(key=plen, reverse=True)
    slot1 = pieces[:N_CORES]
    slot2 = sorted(pieces[N_CORES:], key=plen)      # asc: pair big with small
    slot1 += [None] * (N_CORES - len(slot1))
    slot2 += [None] * (N_CORES - len(slot2))
    # avoid same expert twice on one core (keeps slots independent)
    for i in range(N_CORES):
        if slot1[i] and slot2[i] and slot1[i][0] == slot2[i][0]:
            for j in range(N_CORES):
                if j == i:
                    continue
                ok_j = not (slot1[j] and slot2[i] and slot1[j][0] == slot2[i][0])
                ok_i = not (slot1[i] and slot2[j] and slot1[i][0] == slot2[j][0])
                if ok_j and ok_i:
                    slot2[i], slot2[j] = slot2[j], slot2[i]
                    break
    C1 = _rnd8(max((plen(p) for p in slot1 if p), default=0))
    C2 = _rnd8(max((plen(p) for p in slot2 if p), default=0))
    assert C1 <= 512 and C2 <= 512, (C1, C2)
    return host, slot1, slot2, C1, C2


BODY_VARIANT = "full"          # full | a_only | shared_only


def _build_program(CB, CS, reps=1):
    """One SPMD Tile program shared by all 8 cores. CB/CS: routed slot
    capacities (CS may be 0 to drop the small slot). reps>1 wraps the
    compute in a hardware loop (timing amplification only)."""
    nc = bacc.Bacc(None, target_bir_lowering=False)
    bf = mybir.dt.bfloat16
    f32 = mybir.dt.float32

    slot_caps = [c for c in (CB, CS) if c > 0]
    ns = len(slot_caps)

    # --- I/O ----------------------------------------------------------
    # w13 packed as (g,u) pair panels: one 1MB DMA per SwiGLU pair.
    w13q = [nc.dram_tensor(f"w13q{s}", [NPAIR, P, 2, KH, P], bf,
                           kind="ExternalInput") for s in range(ns)]
    # w2 packed as two consecutive M panels per fetch.
    w2q = [nc.dram_tensor(f"w2q{s}", [MH // 2, P, 2, KI, P], bf,
                          kind="ExternalInput") for s in range(ns)]
    xgq = [nc.dram_tensor(f"xgq{s}", [P, KH, slot_caps[s]], bf, kind="ExternalInput")
           for s in range(ns)]
    wtb = [nc.dram_tensor(f"wtb{s}", [P, slot_caps[s]], f32, kind="ExternalInput")
           for s in range(ns)]
    sguq = nc.dram_tensor("sguq", [P, 2 * KS, KH, P], bf, kind="ExternalInput")
    sdq = nc.dram_tensor("sdq", [P, KS, H], bf, kind="ExternalInput")
    xtq = nc.dram_tensor("xtq", [P, KH, T], bf, kind="ExternalInput")

    yout = [nc.dram_tensor(f"y{s}", [MH // 2, P, 2, slot_caps[s]], bf,
                           kind="ExternalOutput") for s in range(ns)]
    shp = nc.dram_tensor("shp", [MH // 2, P, 2, T], bf, kind="ExternalOutput")

    with tile.TileContext(nc) as tc:
        with (
            tc.tile_pool(name="resident", bufs=1) as res,
            tc.tile_pool(name="wp1", bufs=3) as wp1,
            tc.tile_pool(name="wp2", bufs=3) as wp2,
            tc.tile_pool(name="silu", bufs=3) as spool,
            tc.tile_pool(name="outbuf", bufs=2) as opool,
            tc.tile_pool(name="shoutbuf", bufs=2) as shpool,
            tc.tile_pool(name="psA", bufs=4, space="PSUM") as psA,
            tc.tile_pool(name="psB", bufs=4, space="PSUM") as psB,
        ):
            # Resident activations (loaded once, reused by every rep).
            # DMA issue engines are spread over the three descriptor paths
            # (SP + ACT HWDGE rings, Pool SWDGE) — a single ring serializes
            # transfer completions and becomes the bottleneck.
            xg_t, wt_t, h_t = [], [], []
            for s in range(ns):
                c = slot_caps[s]
                t = res.tile([P, KH, c], bf, name=f"xg{s}_t")
                nc.sync.dma_start(t[:], xgq[s].ap()[:])
                xg_t.append(t)
                w = res.tile([P, c], f32, name=f"wt{s}_t")
                nc.sync.dma_start(w[:], wtb[s].ap()[:])
                wt_t.append(w)
                h_t.append(res.tile([P, KI, c], bf, name=f"h{s}_t"))
            xt_t = res.tile([P, KH, T], bf)
            nc.scalar.dma_start(xt_t[:], xtq.ap()[:])
            sgu_t = res.tile([P, 2 * KS, KH, P], bf)
            nc.gpsimd.dma_start(sgu_t[:], sguq.ap()[:])
            sd_t = res.tile([P, KS, H], bf)   # resident shared-down panels
            nc.gpsimd.dma_start(sd_t[:], sdq.ap()[:])
            hs_t = res.tile([P, KS, T], bf)

            # per-slot DMA issue engine for w13 panels (separate rings)
            g1_dma = [nc.sync, nc.scalar]

            def swiglu(ps_g, ps_u, dst, n):
                # silu(g) * u as sigmoid(g) * g * u (Silu itself is not
                # implemented in CoreSim).
                sg = spool.tile([P, 512], mybir.dt.float32, tag="sg")
                nc.scalar.activation(
                    sg[:, :n], ps_g[:, :n],
                    mybir.ActivationFunctionType.Sigmoid,
                )
                nc.vector.tensor_mul(sg[:, :n], sg[:, :n], ps_g[:, :n])
                nc.vector.tensor_mul(dst, sg[:, :n], ps_u[:, :n])

            def g1_pair(s, pr):
                """Routed GEMM1 pair pr for slot s -> h_t[s][:, pr, :]."""
                cap = slot_caps[s]
                pan = wp1.tile([P, 2, KH, P], bf, tag=f"wpan1_{s}")
                g1_dma[s].dma_start(pan[:], w13q[s].ap()[pr])
                psums = []
                for j in range(2):
                    ps = psA.tile([P, 512], mybir.dt.float32, tag="psA")
                    for k in range(KH):
                        nc.tensor.matmul(
                            ps[:, :cap],
                            lhsT=pan[:, j, k, :],
                            rhs=xg_t[s][:, k, :cap],
                            start=(k == 0),
                            stop=(k == KH - 1),
                        )
                    psums.append(ps)
                swiglu(psums[0], psums[1], h_t[s][:, pr, :cap], cap)

            def g1_shared_unit(ci, pr):
                """Shared GEMM1 (g,u) pair pr on token chunk ci."""
                o = ci * 512
                psums = []
                for j in (pr, pr + KS):
                    ps = psA.tile([P, 512], mybir.dt.float32, tag="psA")
                    for k in range(KH):
                        nc.tensor.matmul(
                            ps[:],
                            lhsT=sgu_t[:, j, k, :],
                            rhs=xt_t[:, k, o:o + 512],
                            start=(k == 0),
                            stop=(k == KH - 1),
                        )
                    psums.append(ps)
                swiglu(psums[0], psums[1], hs_t[:, pr, o:o + 512], 512)

            g2_hold = [{} for _ in range(ns)]

            def g2_slot(s, m):
                cap = slot_caps[s]
                hold = g2_hold[s]
                j = m % 2
                if j == 0:
                    hold["pan"] = wp2.tile([P, 2, KI, P], bf, tag="wpan2",
                                           name=f"w2pan{s}_{m}")
                    nc.gpsimd.dma_start(hold["pan"][:], w2q[s].ap()[m // 2])
                    hold["ot"] = opool.tile([P, 2, cap], bf, tag=f"y{s}",
                                            name=f"yot{s}_{m}")
                pan, ot = hold["pan"], hold["ot"]
                ps = psB.tile([P, 512], mybir.dt.float32, tag="psB")
                for k in range(KI):
                    nc.tensor.matmul(
                        ps[:, :cap],
                        lhsT=pan[:, j, k, :],
                        rhs=h_t[s][:, k, :cap],
                        start=(k == 0),
                        stop=(k == KI - 1),
                    )
                nc.vector.tensor_mul(ot[:, j, :], ps[:, :cap], wt_t[s][:])
                if j == 1:
                    nc.gpsimd.dma_start(yout[s].ap()[m // 2], ot[:])

            sh_hold = {}

            def g2_shared(m):
                j = m % 2
                if j == 0:
                    sh_hold["ot"] = shpool.tile([P, 2, T], bf, tag="sh",
                                                name=f"shot_{m}")
                ot = sh_hold["ot"]
                for ci in range(T // 512):
                    o = ci * 512
                    ps = psA.tile([P, 512], mybir.dt.float32, tag="psA")
                    for k in range(KS):
                        nc.tensor.matmul(
                            ps[:],
                            lhsT=sd_t[:, k, m * P:(m + 1) * P],
                            rhs=hs_t[:, k, o:o + 512],
                            start=(k == 0),
                            stop=(k == KS - 1),
                        )
                    nc.vector.tensor_copy(ot[:, j, o:o + 512], ps[:])
                if j == 1:
                    nc.sync.dma_start(shp.ap()[m // 2], ot[:])

            # shared GEMM1 units interleaved into phase B at these i's
            sh_sched = {2: (0, 0), 4: (0, 1), 6: (0, 2),
                        8: (1, 0), 10: (1, 1), 12: (1, 2)}

            def body():
                if BODY_VARIANT == "a_only":
                    for pr in range(NPAIR):
                        g1_pair(0, pr)
                    return
                if BODY_VARIANT == "shared_only":
                    for u in range(6):
                        g1_shared_unit(u // 3, u % 3)
                    for m in range(MH):
                        g2_shared(m)
                    return
                # Phase A: slot0 GEMM1 (streams its w13 panels)
                for pr in range(NPAIR):
                    g1_pair(0, pr)
                # Phase B: slot1 GEMM1 + slot0 GEMM2 + shared GEMM1
                for i in range(MH):
                    if ns > 1 and i < NPAIR:
                        g1_pair(1, i)
                    g2_slot(0, i)
                    if i in sh_sched:
                        g1_shared_unit(*sh_sched[i])
                # Phase D: shared GEMM2 + slot1 GEMM2
                for m in range(MH):
                    g2_shared(m)
                    if ns > 1:
                        g2_slot(1, m)

            if reps == 1:
                body()
            else:
                with tc.For_i(0, reps, 1,
                              hint_engines=(EngineType.PE, EngineType.SP,
                                            EngineType.DVE, EngineType.Pool,
                                            EngineType.Activation)):
                    body()

    nc.compile()
    return nc


_PROGRAM_CACHE = {}


def _get_program(CB, CS):
    key = (CB, CS)
    if key not in _PROGRAM_CACHE:
        _PROGRAM_CACHE[key] = _build_program(CB, CS)
    return _PROGRAM_CACHE[key]


def _prepare(x, gate_weight, gate_bias, w13, w2, shared_gate_up, shared_down):
    """Host-side routing + packing. Returns (C1, C2, in_maps, meta)."""
    topk_ids, topk_w = _route(x, gate_weight, gate_bias)
    flat_e = topk_ids.ravel()
    flat_w = topk_w.ravel()
    flat_t = np.repeat(np.arange(T, dtype=np.int64), TOP_K)
    idx_e = [flat_t[flat_e == e] for e in range(E)]
    w_e = [flat_w[flat_e == e] for e in range(E)]
    counts = np.array([len(i) for i in idx_e])

    host, slot1, slot2, C1, C2 = _plan(counts)

    # host-computed tiny experts (f32, added during combine)
    extra = None
    for e in host:
        idx = idx_e[e]
        if len(idx) == 0:
            continue
        if extra is None:
            extra = np.zeros((T, H), dtype=F32)
        xe = x[idx].astype(F32)
        gu = xe @ w13[e].astype(F32).T
        g, u = gu[:, :I], gu[:, I:]
        h = (g * _sigmoid(g)) * u
        extra[idx] += (h @ w2[e].astype(F32).T) * w_e[e].astype(F32)[:, None]

    xt_pack = _pack_rhs(x.astype(BF16))                 # [128, KH, T]

    w13_cache, w2_cache = {}, {}

    def packed_w(e):
        if e not in w13_cache:
            a = _pack_lhs_panels(w13[e].astype(BF16), MW, KH)   # [MW,P,KH,P]
            # pair layout [NPAIR, P, 2, KH, P]: (g panel pr, u panel pr+NPAIR)
            w13_cache[e] = np.ascontiguousarray(
                np.stack((a[:NPAIR], a[NPAIR:]), axis=2))
            b = _pack_lhs_panels(w2[e].astype(BF16), MH, KI)    # [MH,P,KI,P]
            w2_cache[e] = np.ascontiguousarray(
                b.reshape(MH // 2, 2, P, KI, P).transpose(0, 2, 1, 3, 4))
        return w13_cache[e], w2_cache[e]

    caps = [c for c in (C1, C2) if c > 0]
    in_maps, meta_cores = [], []
    for c in range(N_CORES):
        im = {}
        cmeta = []
        for s, cap in enumerate(caps):
            piece = (slot1, slot2)[s][c]
            if piece is not None:
                e, lo, hi = piece
                idx = idx_e[e][lo:hi]
                wts = w_e[e][lo:hi]
            else:
                e, idx, wts = 0, np.zeros(0, np.int64), np.zeros(0)
            n = len(idx)
            xg = np.zeros((cap, H), dtype=BF16)
            xg[:n] = x[idx].astype(BF16)
            im[f"xgq{s}"] = _pack_rhs(xg)
            wt = np.zeros((cap,), dtype=F32)
            wt[:n] = wts.astype(F32)
            im[f"wtb{s}"] = np.ascontiguousarray(
                np.broadcast_to(wt[None, :], (P, cap)).astype(F32))
            pw13, pw2 = packed_w(e)
            im[f"w13q{s}"] = pw13
            im[f"w2q{s}"] = pw2
            cmeta.append((s, idx))
        # shared shard: rows [c*352, (c+1)*352) of gate and up, padded to 384
        sh = IS // N_CORES
        lo, hi = c * sh, (c + 1) * sh
        gsl = np.zeros((SSH, H), dtype=F32)
        usl = np.zeros((SSH, H), dtype=F32)
        gsl[:hi - lo] = shared_gate_up[lo:hi]
        usl[:hi - lo] = shared_gate_up[IS + lo:IS + hi]
        sgu_pad = np.concatenate([gsl, usl], 0).astype(BF16)   # [768, H]
        im["sguq"] = np.ascontiguousarray(
            _pack_lhs_panels(sgu_pad, 2 * KS, KH).transpose(1, 0, 2, 3))
        sd_sl = np.zeros((H, SSH), dtype=F32)
        sd_sl[:, :hi - lo] = shared_down[:, lo:hi]
        # resident layout [P, KS, H]: sd[p, k, 128*m + c] = panel[m][p, k, c]
        im["sdq"] = np.ascontiguousarray(
            _pack_lhs_panels(sd_sl.astype(BF16), MH, KS).transpose(1, 2, 0, 3)
            .reshape(P, KS, H))
        im["xtq"] = xt_pack
        in_maps.append(im)
        meta_cores.append(cmeta)
    meta = {"cores": meta_cores, "extra": extra}
    return C1, C2, in_maps, meta


def _combine(results, meta):
    out = np.zeros((H, T), dtype=F32)
    for c in range(N_CORES):
        # shp layout [MH//2, P, 2, T] -> [H, T]
        out += (results[c]["shp"].astype(F32)
                .transpose(0, 2, 1, 3).reshape(H, T))
    out = np.ascontiguousarray(out.T)                   # [T, H]
    for c in range(N_CORES):
        r = results[c]
        for (s, idx) in meta["cores"][c]:
            n = len(idx)
            if n:
                # y layout [MH//2, P, 2, cap] -> [H, cap]
                y = (r[f"y{s}"].astype(F32)
                     .transpose(0, 2, 1, 3).reshape(H, -1))
                out[idx] += y[:, :n].T
    if meta["extra"] is not None:
        out += meta["extra"]
    return out


def kernel(hidden_states, gate_weight, gate_bias, w13, w2,
           shared_gate_up, shared_down):
    x = np.asarray(hidden_states, dtype=F32)
    gate_weight = np.asarray(gate_weight, dtype=F32)
    gate_bias = np.asarray(gate_bias, dtype=F32)
    w13 = np.asarray(w13, dtype=F32)
    w2 = np.asarray(w2, dtype=F32)
    shared_gate_up = np.asarray(shared_gate_up, dtype=F32)
    shared_down = np.asarray(shared_down, dtype=F32)

    CB, CS, in_maps, meta = _prepare(
        x, gate_weight, gate_bias, w13, w2, shared_gate_up, shared_down)
    nc = _get_program(CB, CS)
    res = run_bass_kernel_spmd(nc, in_maps, core_ids=list(range(N_CORES)))
    return _combine(res.results, meta)
